# revision 1
# baseline (speedup 1.0000x reference)
"""Trainium2 Bass kernel: masked multi-head self-attention block.

out = softmax_mask((x @ Wq) (x @ Wk)^T / sqrt(d)) (x @ Wv) @ Wp + b

Sharding: data-parallel over batch B=8 across the 8 NeuronCores (one
batch row per core); weights replicated. Each core computes its batch
fully on-chip; no collectives.

Key compaction: softmax over masked keys is permutation-invariant and
masked keys contribute exactly zero, so each core gathers only the
valid key rows of x (plus padding to a 128 multiple; padded slots get
a -1e30 score bias -> exp = 0). K/V and all attention work then run on
NK ~= 1152 keys instead of 2048. Valid-key indices are computed on the
host from the mask; the row gather itself runs on-device via indirect
DMA.

Per-core dataflow (feature-major / transposed activations):
  x [N,768] --PE-transpose--> X^T [768,N] (f32r); gathered X_c^T too
  Q^T = Wq^T @ X, K^T = Wk^T @ X_c, V = X_c @ Wv (f32r mm, bf16 out)
  per head pair (even head on PE rows/cols 0-63, odd on 64-127),
  per 1024-query half, per 128-key chunk:
    S^T[k,q] = K_j @ Q^T  (bf16, row-group concurrent across the pair)
    P^T = exp(S^T/8 + bias)      -> SBUF bf16 (ScalarE)
    O~^T += V_j^T @ P^T          -> PSUM accum (col-group concurrent)
    rs[:,a] += P^T               -> DVE bf16 partial sums (2x mode)
  denom = ones^T @ rs (PE); O^T = O~^T * (1/denom bcast via DRAM DMA)
  out = O_cat @ Wp + b  (f32r matmuls)
"""
import numpy as np

import concourse.bass as bass
import concourse.tile as tile
from concourse import bacc, mybir
from concourse.bass_utils import run_bass_kernel_spmd
from concourse.masks import make_identity

F32 = mybir.dt.float32
F32R = mybir.dt.float32r
BF16 = mybir.dt.bfloat16
I32 = mybir.dt.int32

B, N, DIM = 8, 2048, 768
H, D = 12, 64
SCALE = D ** -0.5
NCH = N // 128        # 16 token chunks (queries)
KCH = DIM // 128      # 6 feature chunks
QH = 2                # query halves
QW = N // QH          # 1024 queries per half
Exp = mybir.ActivationFunctionType.Exp


def _nslices(w):
    """Split width w into matmul free-dim slices aligned to the 512-f32
    PSUM bank size (one matmul output must stay within one bank)."""
    out = [512] * (w // 512)
    if w % 512:
        out.append(w % 512)
    return out


def _build(nc, tc, aps, nkc):
    x_d, ki_d, kb_d, wqkv_d, wp_d, bp_d, o_d = aps
    NK = nkc * 128

    with tc.tile_pool(name="const", bufs=1) as cpool:
        ident = cpool.tile([128, 128], BF16)
        make_identity(nc, ident)
        ones_r = cpool.tile([128, 1], BF16)
        nc.vector.memset(ones_r, 1.0)
        # compacted-key additive bias (0 valid / -1e30 pad), [128, nkc]
        kb_t = cpool.tile([128, nkc], F32)
        nc.sync.dma_start(out=kb_t, in_=kb_d.rearrange("(j p) -> p j", p=128))
        # gather indices, one column per key chunk
        ki_t = cpool.tile([128, nkc], I32)
        nc.sync.dma_start(out=ki_t, in_=ki_d.rearrange("(j p) -> p j", p=128))
        # b_proj broadcast across partitions
        bp_bc = cpool.tile([128, DIM], F32)
        bp_ap = bass.AP(tensor=bp_d.tensor, offset=bp_d.offset,
                        ap=[[0, 128], list(bp_d.ap[0])])
        nc.sync.dma_start(out=bp_bc, in_=bp_ap)

        with tc.tile_pool(name="qkv_sb", bufs=1) as qkvpool:
            # persistent bf16 Q^T / K^T / V tiles
            qt, kt, v_nat = [], [], []
            for m in range(KCH):
                tq = qkvpool.tile([128, N], BF16, tag=f"qt{m}")
                qt.append(tq)
                tk = qkvpool.tile([128, NK], BF16, tag=f"kt{m}")
                kt.append(tk)
            for t in range(nkc):
                tv = qkvpool.tile([128, DIM], BF16, tag=f"vnat{t}")
                v_nat.append(tv)

            # ---------------- phase A+B: X^T, X_c^T, then QKV ----------
            with tc.tile_pool(name="xt_sb", bufs=1) as xtpool, \
                 tc.tile_pool(name="wv_sb", bufs=1) as wvpool, \
                 tc.tile_pool(name="wqk_sb", bufs=6) as wqkpool, \
                 tc.tile_pool(name="stage_sb", bufs=8) as spool:
                xt, xct = [], []
                for c in range(KCH):
                    t1 = xtpool.tile([128, N], BF16, tag=f"xt{c}")
                    xt.append(t1)
                    t2 = xtpool.tile([128, NK], BF16, tag=f"xct{c}")
                    xct.append(t2)
                with tc.tile_pool(name="ps_a", bufs=4, space="PSUM") as ps_a:
                    # full X^T (for Q)
                    for t_i in range(NCH):
                        x_t = spool.tile([128, DIM], F32, tag="xstage")
                        nc.sync.dma_start(
                            out=x_t, in_=x_d[t_i * 128:(t_i + 1) * 128, :])
                        x_b = spool.tile([128, DIM], BF16, tag="xbstage")
                        nc.scalar.copy(x_b, x_t)
                        for c in range(KCH):
                            tp = ps_a.tile([128, 128], BF16, tag="tp")
                            nc.tensor.transpose(
                                tp, x_b[:, c * 128:(c + 1) * 128], ident)
                            nc.vector.tensor_copy(
                                xt[c][:, t_i * 128:(t_i + 1) * 128], tp)
                    # gathered X_c^T (for K, V)
                    for t_i in range(nkc):
                        x_t = spool.tile([128, DIM], F32, tag="xstage")
                        nc.gpsimd.indirect_dma_start(
                            out=x_t, out_offset=None, in_=x_d,
                            in_offset=bass.IndirectOffsetOnAxis(
                                ap=ki_t[:, t_i:t_i + 1], axis=0))
                        x_b = spool.tile([128, DIM], BF16, tag="xbstage")
                        nc.scalar.copy(x_b, x_t)
                        for c in range(KCH):
                            tp = ps_a.tile([128, 128], BF16, tag="tp")
                            nc.tensor.transpose(
                                tp, x_b[:, c * 128:(c + 1) * 128], ident)
                            nc.vector.tensor_copy(
                                xct[c][:, t_i * 128:(t_i + 1) * 128], tp)

                # resident f32r V-part of w_qkv
                wv = []
                for c in range(KCH):
                    ws = spool.tile([128, DIM], F32, tag="wvstage")
                    nc.sync.dma_start(
                        out=ws,
                        in_=wqkv_d[c * 128:(c + 1) * 128, 2 * DIM:3 * DIM])
                    wr = wvpool.tile([128, DIM], BF16, tag=f"wv{c}")
                    nc.vector.tensor_copy(wr, ws)
                    wv.append(wr)

                # Q^T / K^T; QK weight tiles streamed (each used once)
                with tc.tile_pool(name="ps_qk", bufs=2,
                                  space="PSUM") as ps_qk:
                    for m in range(2 * KCH):
                        is_q = m < KCH
                        src = xt if is_q else xct
                        width = N if is_q else NK
                        mm_ps = ps_qk.tile([128, N], F32,
                                           tag="qk_ps")
                        for c in range(KCH):
                            wqs = wqkpool.tile([128, 128], F32, tag="wqs")
                            nc.sync.dma_start(
                                out=wqs,
                                in_=wqkv_d[c * 128:(c + 1) * 128,
                                           m * 128:(m + 1) * 128])
                            wqr = wqkpool.tile([128, 128], BF16, tag="wqr")
                            nc.vector.tensor_copy(wqr, wqs)
                            off = 0
                            for w in _nslices(width):
                                nc.tensor.matmul(
                                    mm_ps[:, off:off + w],
                                    wqr,
                                    src[c][:, off:off + w],
                                    start=(c == 0), stop=(c == KCH - 1))
                                off += w
                        dst = qt[m] if is_q else kt[m - KCH]
                        nc.scalar.copy(dst, mm_ps[:, 0:width])

                # V natural from gathered rows: V = X_c @ Wv
                with tc.tile_pool(name="ps_v", bufs=2, space="PSUM") as ps_v:
                    for t_i in range(nkc):
                        v_ps = ps_v.tile([128, 2, 512], F32, tag="v_ps")
                        for c in range(KCH):
                            nc.tensor.matmul(
                                v_ps[:, 0, :],
                                xct[c][:, t_i * 128:(t_i + 1) * 128],
                                wv[c][:, 0:512],
                                start=(c == 0), stop=(c == KCH - 1))
                            nc.tensor.matmul(
                                v_ps[:, 1, 0:256],
                                xct[c][:, t_i * 128:(t_i + 1) * 128],
                                wv[c][:, 512:DIM],
                                start=(c == 0), stop=(c == KCH - 1))
                        nc.vector.tensor_copy(
                            v_nat[t_i][:, 0:512], v_ps[:, 0, :])
                        nc.vector.tensor_copy(
                            v_nat[t_i][:, 512:DIM], v_ps[:, 1, 0:256])

            # ---------------- phase C + D ----------------
            with tc.tile_pool(name="ot_sb", bufs=1) as otpool:
                ot = []
                for c in range(KCH):
                    row = []
                    for q in range(QH):
                        t3 = otpool.tile([128, QW], F32R, tag=f"ot{c}_{q}")
                        row.append(t3)
                    ot.append(row)
                with tc.tile_pool(name="wp_sb", bufs=1) as wppool:
                    wp = []
                    for c in range(KCH):
                        ws = wppool.tile([128, DIM], F32, tag=f"wps{c}")
                        nc.sync.dma_start(
                            out=ws, in_=wp_d[c * 128:(c + 1) * 128, :])
                        wr = wppool.tile([128, DIM], F32R, tag=f"wpr{c}")
                        nc.vector.tensor_copy(wr, ws)
                        wp.append(wr)
                    _attention(nc, tc, qt, kt, v_nat, kb_t, ones_r, ot,
                               nkc, wp, bp_bc, o_d)


def _attention(nc, tc, qt, kt, v_nat, kb_t, ones_r, ot, nkc,
               wp, bp_bc, o_d):
    # Head pairs: even head on PE rows/array-cols 0-63, odd on 64-127;
    # QK uses row groups, PV uses col groups -> the pair runs concurrently.
    with tc.tile_pool(name="p_sb", bufs=6) as ppool, \
         tc.tile_pool(name="out_sb", bufs=3) as outpool, \
         tc.tile_pool(name="rs_sb", bufs=3) as rspool, \
         tc.tile_pool(name="ep_sb", bufs=3) as eppool, \
         tc.tile_pool(name="dr_sb", bufs=2, space="DRAM") as drpool, \
         tc.tile_pool(name="ps_s", bufs=2, space="PSUM") as ps_s, \
         tc.tile_pool(name="ps_o", bufs=2, space="PSUM") as ps_o:
        for qh in range(QH):
            q0 = qh * QW
            for hp in range(H // 2):
                kt_c = kt[hp]
                qt_c = qt[hp]
                o_ps = ps_o.tile([128, QW], F32, tag="o_ps")
                rs = rspool.tile([128, 2, QW], BF16, tag="rs")
                for j in range(nkc):
                    p_t = [None, None]
                    for a in range(2):
                        r0 = a * 64
                        s_ps = ps_s.tile([128, QW], F32, tag="s_ps")
                        for g in range(2):
                            nc.tensor.matmul(
                                s_ps[:, g * 512:(g + 1) * 512],
                                kt_c[r0:r0 + 64, j * 128:(j + 1) * 128],
                                qt_c[r0:r0 + 64,
                                     q0 + g * 512:q0 + (g + 1) * 512],
                                start=True, stop=True)
                        pt = ppool.tile([128, QW], BF16, tag="p_t")
                        p_t[a] = pt
                        nc.scalar.activation(pt, s_ps, Exp,
                                             bias=kb_t[:, j:j + 1],
                                             scale=SCALE)
                    for g in range(2):
                        for a in range(2):
                            h = 2 * hp + a
                            nc.tensor.matmul(
                                o_ps[a * 64:(a + 1) * 64,
                                     g * 512:(g + 1) * 512],
                                v_nat[j][:, h * D:(h + 1) * D],
                                p_t[a][:, g * 512:(g + 1) * 512],
                                start=(j == 0), stop=(j == nkc - 1),
                                tile_position=(0, a * 64))
                    for a in range(2):
                        if j == 0:
                            nc.vector.tensor_copy(rs[:, a, :], p_t[a])
                        else:
                            nc.vector.tensor_add(rs[:, a, :], rs[:, a, :],
                                                 p_t[a])
                # denominators: dn[a] = sum over k-partitions of rs[:, a, :]
                b_sb = eppool.tile([128, QW], F32, tag="b_sb")
                for a in range(2):
                    dn_ps = ps_s.tile([1, 2, 512], F32, tag="s_ps")
                    for g in range(2):
                        nc.tensor.matmul(
                            dn_ps[:, g, :], ones_r,
                            rs[:, a, g * 512:(g + 1) * 512],
                            start=True, stop=True)
                    dn_f = eppool.tile([1, QW], F32, tag="dn_f")
                    nc.vector.tensor_copy(
                        dn_f, dn_ps.rearrange("p a b -> p (a b)"))
                    rc_f = eppool.tile([1, QW], F32, tag="rc_f")
                    nc.vector.reciprocal_approx_fast(out=rc_f, in_=dn_f)
                    # broadcast across partitions via DRAM roundtrip
                    rc_dram = drpool.tile([1, QW], F32, tag="rc_dram")
                    nc.sync.dma_start(out=rc_dram, in_=rc_f)
                    rc_bc = bass.AP(tensor=rc_dram.tensor,
                                    offset=rc_dram.offset,
                                    ap=[[0, 64]] + [list(p) for p in
                                                    rc_dram.ap[1:]])
                    nc.sync.dma_start(out=b_sb[a * 64:(a + 1) * 64, :],
                                      in_=rc_bc)
                nc.vector.tensor_mul(
                    ot[hp][qh], o_ps, b_sb)
            # proj for this query half, interleaved with the next half's
            # attention (PSUM drawn from the shared s-pool slots)
            _proj_half(nc, tc, qh, wp, bp_bc, ot, o_d, ps_s, outpool)


def _proj_half(nc, tc, qh, wp, bp_bc, ot, o_d, ps_d, outpool):
    tq = NCH // QH
    for t_i in range(qh * tq, (qh + 1) * tq):
        tl = (t_i % tq) * 128
        pr_ps = ps_d.tile([128, 2, 512], F32, tag="s_ps")
        for c in range(KCH):
            nc.tensor.matmul(
                pr_ps[:, 0, :],
                ot[c][qh][:, tl:tl + 128],
                wp[c][:, 0:512],
                start=(c == 0), stop=(c == KCH - 1))
            nc.tensor.matmul(
                pr_ps[:, 1, 0:256],
                ot[c][qh][:, tl:tl + 128],
                wp[c][:, 512:DIM],
                start=(c == 0), stop=(c == KCH - 1))
        out_t = outpool.tile([128, DIM], F32, tag="out_t")
        nc.vector.tensor_add(
            out_t[:, 0:512], pr_ps[:, 0, :], bp_bc[:, 0:512])
        nc.vector.tensor_add(
            out_t[:, 512:DIM], pr_ps[:, 1, 0:256],
            bp_bc[:, 512:DIM])
        nc.sync.dma_start(
            out=o_d[t_i * 128:(t_i + 1) * 128, :], in_=out_t)


_CACHE = {}


def _get_compiled(nkc):
    if nkc in _CACHE:
        return _CACHE[nkc]
    NK = nkc * 128
    nc = bacc.Bacc("TRN2", target_bir_lowering=False, debug=False,
                   num_devices=B)
    x_d = nc.dram_tensor("x", [N, DIM], F32, kind="ExternalInput").ap()
    ki_d = nc.dram_tensor("kidx", [NK], I32, kind="ExternalInput").ap()
    kb_d = nc.dram_tensor("kbias", [NK], F32, kind="ExternalInput").ap()
    wqkv_d = nc.dram_tensor("w_qkv", [DIM, 3 * DIM], F32,
                            kind="ExternalInput").ap()
    wp_d = nc.dram_tensor("w_proj", [DIM, DIM], F32,
                          kind="ExternalInput").ap()
    bp_d = nc.dram_tensor("b_proj", [DIM], F32, kind="ExternalInput").ap()
    o_d = nc.dram_tensor("out", [N, DIM], F32, kind="ExternalOutput").ap()
    with tile.TileContext(nc) as tc:
        _build(nc, tc, (x_d, ki_d, kb_d, wqkv_d, wp_d, bp_d, o_d), nkc)
    nc.compile()
    _CACHE[nkc] = nc
    return nc


def prep_run(x, mask, w_qkv, w_proj, b_proj):
    """Build the compiled program + per-core input maps (shared with
    test harness for traced runs)."""
    x = np.ascontiguousarray(np.asarray(x, dtype=np.float32))
    mask = np.ascontiguousarray(np.asarray(mask, dtype=np.int32))
    w_qkv = np.ascontiguousarray(np.asarray(w_qkv, dtype=np.float32))
    w_proj = np.ascontiguousarray(np.asarray(w_proj, dtype=np.float32))
    b_proj = np.ascontiguousarray(np.asarray(b_proj, dtype=np.float32))

    # host-side compaction metadata: indices of valid keys per batch
    idxs = [np.flatnonzero(mask[b]).astype(np.int32) for b in range(B)]
    max_valid = max(len(i) for i in idxs)
    nkc = min(NCH, max(1, -(-max_valid // 128)))
    NK = nkc * 128
    kidx = np.zeros((B, NK), dtype=np.int32)
    kbias = np.full((B, NK), -1.0e30, dtype=np.float32)
    for b in range(B):
        n = len(idxs[b])
        kidx[b, :n] = idxs[b]
        kbias[b, :n] = 0.0

    nc = _get_compiled(nkc)
    in_maps = [
        {"x": x[b], "kidx": kidx[b], "kbias": kbias[b], "w_qkv": w_qkv,
         "w_proj": w_proj, "b_proj": b_proj}
        for b in range(B)
    ]
    return nc, in_maps


def kernel(x, mask, w_qkv, w_proj, b_proj):
    nc, in_maps = prep_run(x, mask, w_qkv, w_proj, b_proj)
    last_err = None
    for _ in range(3):
        try:
            res = run_bass_kernel_spmd(nc, in_maps, list(range(B))).results
            return np.stack([res[b]["out"] for b in range(B)], axis=0)
        except Exception as e:  # transient device hiccup: retry
            last_err = e
    raise last_err



# revision 12
# speedup vs baseline: 1.1651x; 1.1651x over previous
"""Trainium2 Bass kernel: masked multi-head self-attention block.

out = softmax_mask((x @ Wq) (x @ Wk)^T / sqrt(d)) (x @ Wv) @ Wp + b

Sharding: data-parallel over batch B=8 across the 8 NeuronCores (one
batch row per core); weights replicated. Each core computes its batch
fully on-chip; no collectives.

Key compaction: softmax over masked keys is permutation-invariant and
masked keys contribute exactly zero, so each core gathers only the
valid key rows of x (plus padding to a 128 multiple; padded slots get
a -1e30 score bias -> exp = 0). K/V and all attention work then run on
NK ~= 1152 keys instead of 2048.

v2 dataflow (vs baseline):
  - X^T / X_c^T produced entirely by DMA: x is cast f32->bf16 into a
    DRAM scratch (SWDGE cast-DMA), then xbar DMA-transposed straight
    into SBUF. The gathered rows are fetched by indirect DMA, cast on
    DVE, written back to DRAM and DMA-transposed too. No PE transposes.
  - Attention runs on 512-query blocks (QH=4): per (qblock, headpair,
    keychunk j): two S matmuls (row-group packed pair), ONE merged exp
    [128, 2, 512] on ScalarE, one merged running-sum add on DVE, two PV
    matmuls (col-group packed). The j-loop is software-pipelined: S for
    j+2 is issued before PV for j so the PE never head-of-line blocks
    on the exp, and the PE stream stays dense (HAM stays at 2.4 GHz).
  - Denominators: ones^T @ rs matmul, reciprocal AFTER partition
    broadcast (one [128,512] reciprocal instead of two [1,512]).
  - proj matmuls are drained into the NEXT query-block's attention
    loop (2 matmuls per j-step) from a backlog, so they fill PE slack
    instead of stalling the exp pipeline.
"""
import numpy as np

import concourse.bass as bass
import concourse.tile as tile
from concourse import bacc, mybir
from concourse.bass_utils import run_bass_kernel_spmd

F32 = mybir.dt.float32
BF16 = mybir.dt.bfloat16
I32 = mybir.dt.int32

B, N, DIM = 8, 2048, 768
H, D = 12, 64
SCALE = D ** -0.5
NCH = N // 128        # 16 token chunks
KCH = DIM // 128      # 6 feature chunks
QH = 4                # query blocks
QW = N // QH          # 512 queries per block
Exp = mybir.ActivationFunctionType.Exp


def _nslices(w):
    out = [512] * (w // 512)
    if w % 512:
        out.append(w % 512)
    return out


def _build(nc, tc, aps, nkc):
    x_d, ki_d, kb_d, wqkv_d, wp_d, bp_d, o_d = aps
    NK = nkc * 128

    cpool = tc.alloc_tile_pool(name="const", bufs=1)
    ones_r = cpool.tile([128, 1], BF16)
    nc.vector.memset(ones_r, 1.0)
    kb_t = cpool.tile([128, nkc], F32)
    nc.sync.dma_start(out=kb_t, in_=kb_d.rearrange("(j p) -> p j", p=128))
    ki_t = cpool.tile([128, nkc], I32)
    nc.sync.dma_start(out=ki_t, in_=ki_d.rearrange("(j p) -> p j", p=128))
    bp_bc = cpool.tile([128, DIM], F32)
    bp_ap = bass.AP(tensor=bp_d.tensor, offset=bp_d.offset,
                    ap=[[0, 128], list(bp_d.ap[0])])
    nc.sync.dma_start(out=bp_bc, in_=bp_ap)

    # DRAM scratch
    drbig = tc.alloc_tile_pool(name="dr_big", bufs=1, space="DRAM")
    x_bf = drbig.tile([N, DIM], BF16)
    xc_bf = drbig.tile([NK, DIM], BF16)

    # persistent bf16 tiles
    qkvpool = tc.alloc_tile_pool(name="qkv_sb", bufs=1)
    qt = [qkvpool.tile([128, N], BF16, tag=f"qt{m}", name=f"qt{m}")
          for m in range(KCH)]
    kt = [qkvpool.tile([128, NK], BF16, tag=f"kt{m}", name=f"kt{m}")
          for m in range(KCH)]
    v_nat = [qkvpool.tile([128, DIM], BF16, tag=f"vn{t}", name=f"vn{t}")
             for t in range(nkc)]
    wp_sb = [qkvpool.tile([128, DIM], BF16, tag=f"wp{c}", name=f"wp{c}")
             for c in range(KCH)]
    ot = [[qkvpool.tile([128, QW], BF16, tag=f"ot{c}_{q}", name=f"ot{c}_{q}")
           for q in range(QH)] for c in range(KCH)]

    # ---------------- phase A: DMA transposes + gathers -------------
    with tc.tile_pool(name="xt_sb", bufs=1) as xtpool, \
         tc.tile_pool(name="wq_sb", bufs=1) as wqpool, \
         tc.tile_pool(name="stage_sb", bufs=3) as spool:
        # gathered rows first (f32, no dependency on the cast)
        xcb = []
        for t in range(nkc):
            xg = spool.tile([128, DIM], F32, tag="xg")
            nc.gpsimd.indirect_dma_start(
                out=xg, out_offset=None, in_=x_d,
                in_offset=bass.IndirectOffsetOnAxis(
                    ap=ki_t[:, t:t + 1], axis=0))
            xb = spool.tile([128, DIM], BF16, tag="xb")
            nc.vector.tensor_copy(xb, xg)
            xcb.append(xb)
        # x cast f32 -> bf16 (DRAM -> DRAM, SWDGE)
        nc.gpsimd.dma_start(out=x_bf[:, :], in_=x_d)
        # write gathered bf16 rows back to DRAM (scalar HWDGE queue)
        for t in range(nkc):
            nc.scalar.dma_start(out=xc_bf[t * 128:(t + 1) * 128, :],
                                in_=xcb[t])
        # xbar transposes (sync HWDGE queue)
        xt, xct = [], []
        for c in range(KCH):
            t2 = xtpool.tile([128, NK], BF16, tag=f"xct{c}")
            nc.sync.dma_start(out=t2, in_=xc_bf[:, c * 128:(c + 1) * 128],
                              transpose=True)
            xct.append(t2)
        for c in range(KCH):
            t1 = xtpool.tile([128, N], BF16, tag=f"xt{c}")
            nc.sync.dma_start(out=t1, in_=x_bf[:, c * 128:(c + 1) * 128],
                              transpose=True)
            xt.append(t1)

        # weights: f32 load (scalar queue) + DVE cast to resident bf16
        wq_sb = []
        for c in range(KCH):
            ws = spool.tile([128, 3 * DIM], F32, tag="wstage")
            nc.scalar.dma_start(out=ws, in_=wqkv_d[c * 128:(c + 1) * 128, :])
            wr = wqpool.tile([128, 3 * DIM], BF16, tag=f"wq{c}")
            nc.vector.tensor_copy(wr, ws)
            wq_sb.append(wr)
        for c in range(KCH):
            ws = spool.tile([128, DIM], F32, tag="wpstage")
            nc.scalar.dma_start(out=ws, in_=wp_d[c * 128:(c + 1) * 128, :])
            nc.vector.tensor_copy(wp_sb[c], ws)

        # ---------------- phase B: K^T, V, Q^T ----------------------
        with tc.tile_pool(name="ps_k", bufs=2, space="PSUM") as ps_k:
            # K^T chunks (need xct only)
            for m in range(KCH):
                mm_ps = ps_k.tile([128, NK], F32, tag="k_ps")
                for c in range(KCH):
                    off = 0
                    for w in _nslices(NK):
                        nc.tensor.matmul(
                            mm_ps[:, off:off + w],
                            wq_sb[c][:, DIM + m * 128:DIM + (m + 1) * 128],
                            xct[c][:, off:off + w],
                            start=(c == 0), stop=(c == KCH - 1))
                        off += w
                if m % 2 == 0:
                    nc.scalar.copy(kt[m], mm_ps)
                else:
                    nc.vector.tensor_copy(kt[m], mm_ps)
        # V natural
        with tc.tile_pool(name="ps_v", bufs=2, space="PSUM") as ps_v:
            for t in range(nkc):
                v_ps = ps_v.tile([128, 2, 512], F32, tag="v_ps")
                for c in range(KCH):
                    nc.tensor.matmul(
                        v_ps[:, 0, :],
                        xct[c][:, t * 128:(t + 1) * 128],
                        wq_sb[c][:, 2 * DIM:2 * DIM + 512],
                        start=(c == 0), stop=(c == KCH - 1))
                    nc.tensor.matmul(
                        v_ps[:, 1, 0:256],
                        xct[c][:, t * 128:(t + 1) * 128],
                        wq_sb[c][:, 2 * DIM + 512:3 * DIM],
                        start=(c == 0), stop=(c == KCH - 1))
                nc.vector.tensor_copy(v_nat[t][:, 0:512], v_ps[:, 0, :])
                nc.vector.tensor_copy(v_nat[t][:, 512:DIM],
                                      v_ps[:, 1, 0:256])
        # Q^T chunks
        with tc.tile_pool(name="ps_q", bufs=2, space="PSUM") as ps_q:
            for m in range(KCH):
                mm_ps = ps_q.tile([128, N], F32, tag="q_ps")
                for c in range(KCH):
                    for g in range(N // 512):
                        nc.tensor.matmul(
                            mm_ps[:, g * 512:(g + 1) * 512],
                            wq_sb[c][:, m * 128:(m + 1) * 128],
                            xt[c][:, g * 512:(g + 1) * 512],
                            start=(c == 0), stop=(c == KCH - 1))
                if m % 2 == 0:
                    nc.scalar.copy(qt[m], mm_ps)
                else:
                    nc.vector.tensor_copy(qt[m], mm_ps)

    # ---------------- phase C: attention + proj ---------------------
    _attention(nc, tc, qt, kt, v_nat, kb_t, ones_r, ot, nkc, wp_sb,
               bp_bc, o_d)
    qkvpool.release()
    drbig.release()
    cpool.release()


def _attention(nc, tc, qt, kt, v_nat, kb_t, ones_r, ot, nkc,
               wp, bp_bc, o_d):
    with tc.tile_pool(name="p_sb", bufs=3) as ppool, \
         tc.tile_pool(name="rs_sb", bufs=2) as rspool, \
         tc.tile_pool(name="ep_sb", bufs=3) as eppool, \
         tc.tile_pool(name="out_sb", bufs=3) as outpool, \
         tc.tile_pool(name="dr_sb", bufs=3, space="DRAM") as drpool, \
         tc.tile_pool(name="ps_c", bufs=1, space="PSUM") as ps:

        backlog = []

        def drain(k):
            for _ in range(min(k, len(backlog))):
                backlog.pop(0)()

        def emit_S(qh, hp, j):
            q0 = qh * QW
            s_t = ps.tile([128, 2, 512], F32, tag="s", bufs=2, name="s_t")
            for a in range(2):
                r0 = a * 64
                nc.tensor.matmul(
                    s_t[:, a, :],
                    kt[hp][r0:r0 + 64, j * 128:(j + 1) * 128],
                    qt[hp][r0:r0 + 64, q0:q0 + QW],
                    start=True, stop=True)
            return s_t

        def queue_proj(qh):
            # proj for query block qh: 4 token chunks of 128
            for ti in range(4):
                t_i = qh * 4 + ti
                st = {}

                def start_chunk(t_i=t_i, st=st):
                    st["pr"] = ps.tile([128, 2, 512], F32, tag="pr", bufs=1,
                                       name="pr")

                def cstep(c, t_i=t_i, st=st):
                    tl = (t_i % 4) * 128
                    pr = st["pr"]
                    nc.tensor.matmul(
                        pr[:, 0, :], ot[c][t_i // 4][:, tl:tl + 128],
                        wp[c][:, 0:512],
                        start=(c == 0), stop=(c == KCH - 1))
                    nc.tensor.matmul(
                        pr[:, 1, 0:256], ot[c][t_i // 4][:, tl:tl + 128],
                        wp[c][:, 512:DIM],
                        start=(c == 0), stop=(c == KCH - 1))

                def finish(t_i=t_i, st=st):
                    pr = st["pr"]
                    out_t = outpool.tile([128, DIM], F32, tag="out_t",
                                         name="out_t")
                    nc.vector.tensor_add(out_t[:, 0:512], pr[:, 0, :],
                                         bp_bc[:, 0:512])
                    nc.vector.tensor_add(out_t[:, 512:DIM], pr[:, 1, 0:256],
                                         bp_bc[:, 512:DIM])
                    nc.sync.dma_start(
                        out=o_d[t_i * 128:(t_i + 1) * 128, :], in_=out_t)

                backlog.append(start_chunk)
                for c in range(KCH):
                    backlog.append(lambda c=c, f=cstep: f(c))
                backlog.append(finish)

        for qh in range(QH):
            for hp in range(H // 2):
                s_pend = [emit_S(qh, hp, 0), emit_S(qh, hp, 1)]
                o_t = ps.tile([128, QW], F32, tag="o", bufs=2)
                rs_t = rspool.tile([128, 2, 512], BF16, tag="rs")
                for j in range(nkc):
                    s_t = s_pend[j]
                    pt_t = ppool.tile([128, 2, 512], BF16, tag="pt")
                    nc.scalar.activation(pt_t, s_t, Exp,
                                         bias=kb_t[:, j:j + 1], scale=SCALE)
                    if j + 2 < nkc:
                        s_pend.append(emit_S(qh, hp, j + 2))
                    if j == 0:
                        nc.vector.tensor_copy(rs_t, pt_t)
                    else:
                        nc.vector.tensor_add(rs_t, rs_t, pt_t)
                    for a in range(2):
                        h = 2 * hp + a
                        nc.tensor.matmul(
                            o_t[a * 64:(a + 1) * 64, :],
                            v_nat[j][:, h * D:(h + 1) * D],
                            pt_t[:, a, :],
                            start=(j == 0), stop=(j == nkc - 1),
                            tile_position=(0, a * 64))
                    drain(2)
                # denominators: dn[a] = ones^T @ rs[:, a, :]
                dn_t = ps.tile([1, 2, 512], F32, tag="s", bufs=2)
                for a in range(2):
                    nc.tensor.matmul(dn_t[:, a, :], ones_r, rs_t[:, a, :],
                                     start=True, stop=True)
                dn_sb = eppool.tile([1, 2, 512], F32, tag="dn_sb")
                nc.vector.tensor_copy(dn_sb, dn_t)
                rc_dram = drpool.tile([1024], F32, tag="rc_dram")
                nc.sync.dma_start(out=rc_dram, in_=dn_sb)
                b_raw = eppool.tile([128, QW], F32, tag="b_raw")
                for a in range(2):
                    bc_ap = bass.AP(
                        tensor=rc_dram.tensor,
                        offset=rc_dram.offset + a * 512,
                        ap=[[0, 64], [1, 512]])
                    nc.sync.dma_start(out=b_raw[a * 64:(a + 1) * 64, :],
                                      in_=bc_ap)
                rc_b = eppool.tile([128, QW], F32, tag="rc_b")
                nc.vector.reciprocal_approx_fast(out=rc_b, in_=b_raw)
                nc.vector.tensor_mul(ot[hp][qh], o_t, rc_b)
            queue_proj(qh)
        drain(len(backlog))


_CACHE = {}


def _get_compiled(nkc):
    if nkc in _CACHE:
        return _CACHE[nkc]
    NK = nkc * 128
    nc = bacc.Bacc("TRN2", target_bir_lowering=False, debug=False,
                   num_devices=B)
    x_d = nc.dram_tensor("x", [N, DIM], F32, kind="ExternalInput").ap()
    ki_d = nc.dram_tensor("kidx", [NK], I32, kind="ExternalInput").ap()
    kb_d = nc.dram_tensor("kbias", [NK], F32, kind="ExternalInput").ap()
    wqkv_d = nc.dram_tensor("w_qkv", [DIM, 3 * DIM], F32,
                            kind="ExternalInput").ap()
    wp_d = nc.dram_tensor("w_proj", [DIM, DIM], F32,
                          kind="ExternalInput").ap()
    bp_d = nc.dram_tensor("b_proj", [DIM], F32, kind="ExternalInput").ap()
    o_d = nc.dram_tensor("out", [N, DIM], F32, kind="ExternalOutput").ap()
    with tile.TileContext(nc) as tc:
        _build(nc, tc, (x_d, ki_d, kb_d, wqkv_d, wp_d, bp_d, o_d), nkc)
    nc.compile()
    _CACHE[nkc] = nc
    return nc


def prep_run(x, mask, w_qkv, w_proj, b_proj):
    """Build the compiled program + per-core input maps."""
    x = np.ascontiguousarray(np.asarray(x, dtype=np.float32))
    mask = np.ascontiguousarray(np.asarray(mask, dtype=np.int32))
    w_qkv = np.ascontiguousarray(np.asarray(w_qkv, dtype=np.float32))
    w_proj = np.ascontiguousarray(np.asarray(w_proj, dtype=np.float32))
    b_proj = np.ascontiguousarray(np.asarray(b_proj, dtype=np.float32))

    idxs = [np.flatnonzero(mask[b]).astype(np.int32) for b in range(B)]
    max_valid = max(len(i) for i in idxs)
    nkc = min(NCH, max(1, -(-max_valid // 128)))
    NK = nkc * 128
    kidx = np.zeros((B, NK), dtype=np.int32)
    kbias = np.full((B, NK), -1.0e30, dtype=np.float32)
    for b in range(B):
        n = len(idxs[b])
        kidx[b, :n] = idxs[b]
        kbias[b, :n] = 0.0

    nc = _get_compiled(nkc)
    in_maps = [
        {"x": x[b], "kidx": kidx[b], "kbias": kbias[b], "w_qkv": w_qkv,
         "w_proj": w_proj, "b_proj": b_proj}
        for b in range(B)
    ]
    return nc, in_maps


def kernel(x, mask, w_qkv, w_proj, b_proj):
    nc, in_maps = prep_run(x, mask, w_qkv, w_proj, b_proj)
    last_err = None
    for _ in range(3):
        try:
            res = run_bass_kernel_spmd(nc, in_maps, list(range(B))).results
            return np.stack([res[b]["out"] for b in range(B)], axis=0)
        except Exception as e:  # transient device hiccup: retry
            last_err = e
    raise last_err


# revision 14
# speedup vs baseline: 1.3993x; 1.2010x over previous
"""Trainium2 Bass kernel: masked multi-head self-attention block.

out = softmax_mask((x @ Wq) (x @ Wk)^T / sqrt(d)) (x @ Wv) @ Wp + b

Sharding: data-parallel over batch B=8 across the 8 NeuronCores (one
batch row per core); weights replicated; no collectives.

Key compaction: masked keys contribute exactly zero, so each core
gathers only the valid key rows of x (padded to a 128 multiple; padded
slots get a -1e30 score bias -> exp = 0). K/V and attention run on
NK ~= 1152 keys instead of 2048.

v3 structure:
  - Phase A: x chunks + gathered chunks stream in (f32, HWDGE/SWDGE),
    are cast to bf16 (ScalarE/DVE alternating) and PE-transposed into
    X^T / X_c^T (DVE evacuates PSUM). Weights stream in f32 and are
    cast to resident bf16 on DVE.
  - Phase B: K^T, V, Q^T via bf16 matmuls with resident weights.
  - Phase C attention runs as ONE flat software-pipelined stream over
    (qblock, headpair, keychunk) steps with S-matmul lookahead of 2
    steps crossing all boundaries, so neither the PE nor ScalarE ever
    head-of-line blocks: per step two S matmuls (row-packed pair), one
    merged exp [128,2,512] (ScalarE), one merged running-sum add
    (DVE), two PV matmuls (col-packed pair). Denominator matmuls land
    *after* the next pair's S matmuls; reciprocal happens after the
    DRAM broadcast (one [128,512] reciprocal per pair). proj matmuls
    drain from a backlog, 2 per step, filling PE slack.
"""
import numpy as np

import concourse.bass as bass
import concourse.tile as tile
from concourse import bacc, mybir
from concourse.bass_utils import run_bass_kernel_spmd
from concourse.masks import make_identity

F32 = mybir.dt.float32
BF16 = mybir.dt.bfloat16
I32 = mybir.dt.int32

B, N, DIM = 8, 2048, 768
H, D = 12, 64
SCALE = D ** -0.5
NCH = N // 128        # 16 token chunks
KCH = DIM // 128      # 6 feature chunks
QH = 4                # query blocks
QW = N // QH          # 512 queries per block
Exp = mybir.ActivationFunctionType.Exp


def _nslices(w):
    out = [512] * (w // 512)
    if w % 512:
        out.append(w % 512)
    return out


def _build(nc, tc, aps, nkc):
    x_d, ki_d, kb_d, wqkv_d, wp_d, bp_d, o_d = aps
    NK = nkc * 128

    cpool = tc.alloc_tile_pool(name="const", bufs=1)
    ident = cpool.tile([128, 128], BF16)
    make_identity(nc, ident)
    ones_r = cpool.tile([128, 1], BF16)
    nc.vector.memset(ones_r, 1.0)
    kb_t = cpool.tile([128, nkc], F32)
    nc.sync.dma_start(out=kb_t, in_=kb_d.rearrange("(j p) -> p j", p=128))
    ki_t = cpool.tile([128, nkc], I32)
    nc.sync.dma_start(out=ki_t, in_=ki_d.rearrange("(j p) -> p j", p=128))
    bp_bc = cpool.tile([128, DIM], F32)
    bp_ap = bass.AP(tensor=bp_d.tensor, offset=bp_d.offset,
                    ap=[[0, 128], list(bp_d.ap[0])])
    nc.sync.dma_start(out=bp_bc, in_=bp_ap)

    # persistent bf16 tiles
    qkvpool = tc.alloc_tile_pool(name="qkv_sb", bufs=1)
    qt = [qkvpool.tile([128, N], BF16, tag=f"qt{m}", name=f"qt{m}")
          for m in range(KCH)]
    kt = [qkvpool.tile([128, NK], BF16, tag=f"kt{m}", name=f"kt{m}")
          for m in range(KCH)]
    v_nat = [qkvpool.tile([128, DIM], BF16, tag=f"vn{t}", name=f"vn{t}")
             for t in range(nkc)]
    wp_sb = [qkvpool.tile([128, DIM], BF16, tag=f"wp{c}", name=f"wp{c}")
             for c in range(KCH)]
    ot = [[qkvpool.tile([128, QW], BF16, tag=f"ot{c}_{q}", name=f"ot{c}_{q}")
           for q in range(QH)] for c in range(KCH)]

    # ---------------- phase A: loads, casts, PE transposes ----------
    with tc.tile_pool(name="xt_sb", bufs=1) as xtpool, \
         tc.tile_pool(name="wq_sb", bufs=1) as wqpool, \
         tc.tile_pool(name="stage_sb", bufs=4) as spool:
        xt = [xtpool.tile([128, N], BF16, tag=f"xt{c}", name=f"xt{c}")
              for c in range(KCH)]
        xct = [xtpool.tile([128, NK], BF16, tag=f"xct{c}", name=f"xct{c}")
               for c in range(KCH)]

        # gathers first on the gpsimd queue, x loads on scalar queue
        gath = []
        for t in range(nkc):
            xg = spool.tile([128, DIM], F32, tag="xg", name=f"xg{t}",
                            bufs=3)
            nc.gpsimd.indirect_dma_start(
                out=xg, out_offset=None, in_=x_d,
                in_offset=bass.IndirectOffsetOnAxis(
                    ap=ki_t[:, t:t + 1], axis=0))
            gath.append(xg)
        loads = []
        for ti in range(NCH):
            xs = spool.tile([128, DIM], F32, tag="xs", name=f"xs{ti}",
                            bufs=3)
            nc.scalar.dma_start(out=xs,
                                in_=x_d[ti * 128:(ti + 1) * 128, :])
            loads.append(xs)
        # weight loads queue up behind the x loads on the scalar queue
        wstage = []
        for c in range(KCH):
            ws = spool.tile([128, 3 * DIM], F32, tag="wstage",
                            name=f"ws{c}", bufs=2)
            nc.scalar.dma_start(out=ws, in_=wqkv_d[c * 128:(c + 1) * 128, :])
            wstage.append(ws)
        # interleave gathered + full chunks for transpose (arrival order)
        order = []
        for i in range(NCH):
            if i < nkc:
                order.append((gath[i], xct, i))
            order.append((loads[i], xt, i))

        wq_sb = [wqpool.tile([128, 3 * DIM], BF16, tag=f"wq{c}",
                             name=f"wq{c}") for c in range(KCH)]
        with tc.tile_pool(name="ps_a", bufs=6, space="PSUM") as ps_a:
            for i, (src, dst, col) in enumerate(order):
                xb = spool.tile([128, DIM], BF16, tag="xb", name=f"xb{i}",
                                bufs=3)
                nc.scalar.copy(xb, src)
                for c in range(KCH):
                    tp = ps_a.tile([128, 128], BF16, tag="tp", name="tp")
                    nc.tensor.transpose(
                        tp, xb[:, c * 128:(c + 1) * 128], ident)
                    nc.vector.tensor_copy(
                        dst[c][:, col * 128:(col + 1) * 128], tp)
                # spread the resident-weight casts through the stream
                if i % 4 == 3 and i // 4 < KCH:
                    c = i // 4
                    nc.vector.tensor_copy(wq_sb[c], wstage[c])

        for c in range(KCH):
            ws = spool.tile([128, DIM], F32, tag="wpstage", name=f"wps{c}",
                            bufs=2)
            nc.scalar.dma_start(out=ws, in_=wp_d[c * 128:(c + 1) * 128, :])
            nc.vector.tensor_copy(wp_sb[c], ws)

        # ---------------- phase B: K^T, V, Q^T ----------------------
        with tc.tile_pool(name="ps_k", bufs=2, space="PSUM") as ps_k:
            for m in range(KCH):
                mm_ps = ps_k.tile([128, NK], F32, tag="k_ps", name="k_ps")
                for c in range(KCH):
                    off = 0
                    for w in _nslices(NK):
                        nc.tensor.matmul(
                            mm_ps[:, off:off + w],
                            wq_sb[c][:, DIM + m * 128:DIM + (m + 1) * 128],
                            xct[c][:, off:off + w],
                            start=(c == 0), stop=(c == KCH - 1))
                        off += w
                if m % 2 == 0:
                    nc.scalar.copy(kt[m], mm_ps)
                else:
                    nc.vector.tensor_copy(kt[m], mm_ps)
        with tc.tile_pool(name="ps_v", bufs=2, space="PSUM") as ps_v:
            for t in range(nkc):
                v_ps = ps_v.tile([128, 2, 512], F32, tag="v_ps", name="v_ps")
                for c in range(KCH):
                    nc.tensor.matmul(
                        v_ps[:, 0, :],
                        xct[c][:, t * 128:(t + 1) * 128],
                        wq_sb[c][:, 2 * DIM:2 * DIM + 512],
                        start=(c == 0), stop=(c == KCH - 1))
                    nc.tensor.matmul(
                        v_ps[:, 1, 0:256],
                        xct[c][:, t * 128:(t + 1) * 128],
                        wq_sb[c][:, 2 * DIM + 512:3 * DIM],
                        start=(c == 0), stop=(c == KCH - 1))
                nc.vector.tensor_copy(v_nat[t][:, 0:512], v_ps[:, 0, :])
                nc.vector.tensor_copy(v_nat[t][:, 512:DIM],
                                      v_ps[:, 1, 0:256])
        with tc.tile_pool(name="ps_q", bufs=2, space="PSUM") as ps_q:
            for m in range(KCH):
                mm_ps = ps_q.tile([128, N], F32, tag="q_ps", name="q_ps")
                for c in range(KCH):
                    for g in range(N // 512):
                        nc.tensor.matmul(
                            mm_ps[:, g * 512:(g + 1) * 512],
                            wq_sb[c][:, m * 128:(m + 1) * 128],
                            xt[c][:, g * 512:(g + 1) * 512],
                            start=(c == 0), stop=(c == KCH - 1))
                if m % 2 == 0:
                    nc.scalar.copy(qt[m], mm_ps)
                else:
                    nc.vector.tensor_copy(qt[m], mm_ps)

    # ---------------- phase C: attention + proj ---------------------
    _attention(nc, tc, qt, kt, v_nat, kb_t, ones_r, ot, nkc, wp_sb,
               bp_bc, o_d)
    qkvpool.release()
    cpool.release()


def _attention(nc, tc, qt, kt, v_nat, kb_t, ones_r, ot, nkc,
               wp, bp_bc, o_d):
    with tc.tile_pool(name="p_sb", bufs=3) as ppool, \
         tc.tile_pool(name="rs_sb", bufs=2) as rspool, \
         tc.tile_pool(name="ep_sb", bufs=3) as eppool, \
         tc.tile_pool(name="out_sb", bufs=3) as outpool, \
         tc.tile_pool(name="dr_sb", bufs=3, space="DRAM") as drpool, \
         tc.tile_pool(name="ps_c", bufs=1, space="PSUM") as ps:

        backlog = []

        def drain(k):
            for _ in range(min(k, len(backlog))):
                backlog.pop(0)()

        def emit_S(qh, hp, j):
            q0 = qh * QW
            s_t = ps.tile([128, 2, 512], F32, tag="s", bufs=2, name="s_t")
            for a in range(2):
                r0 = a * 64
                nc.tensor.matmul(
                    s_t[:, a, :],
                    kt[hp][r0:r0 + 64, j * 128:(j + 1) * 128],
                    qt[hp][r0:r0 + 64, q0:q0 + QW],
                    start=True, stop=True)
            return s_t

        def queue_proj(qh):
            for ti in range(4):
                t_i = qh * 4 + ti
                st = {}

                def start_chunk(t_i=t_i, st=st):
                    st["pr"] = ps.tile([128, 2, 512], F32, tag="pr", bufs=1,
                                       name="pr")

                def cstep(c, t_i=t_i, st=st):
                    tl = (t_i % 4) * 128
                    pr = st["pr"]
                    nc.tensor.matmul(
                        pr[:, 0, :], ot[c][t_i // 4][:, tl:tl + 128],
                        wp[c][:, 0:512],
                        start=(c == 0), stop=(c == KCH - 1))
                    nc.tensor.matmul(
                        pr[:, 1, 0:256], ot[c][t_i // 4][:, tl:tl + 128],
                        wp[c][:, 512:DIM],
                        start=(c == 0), stop=(c == KCH - 1))

                def finish(t_i=t_i, st=st):
                    pr = st["pr"]
                    out_t = outpool.tile([128, DIM], F32, tag="out_t",
                                         name="out_t")
                    nc.vector.tensor_add(out_t[:, 0:512], pr[:, 0, :],
                                         bp_bc[:, 0:512])
                    nc.vector.tensor_add(out_t[:, 512:DIM], pr[:, 1, 0:256],
                                         bp_bc[:, 512:DIM])
                    nc.sync.dma_start(
                        out=o_d[t_i * 128:(t_i + 1) * 128, :], in_=out_t)

                backlog.append(start_chunk)
                for c in range(KCH):
                    backlog.append(lambda c=c, f=cstep: f(c))
                backlog.append(finish)

        def epilogue(qh, hp, rs_t, o_t):
            # dn[a] = ones^T @ rs[:, a, :]; reciprocal after broadcast
            dn_t = ps.tile([1, 2, 512], F32, tag="s", bufs=2, name="dn_t")
            for a in range(2):
                nc.tensor.matmul(dn_t[:, a, :], ones_r, rs_t[:, a, :],
                                 start=True, stop=True)
            dn_sb = eppool.tile([1, 2, 512], F32, tag="dn_sb", name="dn_sb")
            nc.vector.tensor_copy(dn_sb, dn_t)
            rc_dram = drpool.tile([1024], F32, tag="rc_dram", name="rc_dram")
            nc.sync.dma_start(out=rc_dram, in_=dn_sb)
            b_raw = eppool.tile([128, QW], F32, tag="b_raw", name="b_raw")
            for a in range(2):
                bc_ap = bass.AP(
                    tensor=rc_dram.tensor,
                    offset=rc_dram.offset + a * 512,
                    ap=[[0, 64], [1, 512]])
                nc.sync.dma_start(out=b_raw[a * 64:(a + 1) * 64, :],
                                  in_=bc_ap)
            rc_b = eppool.tile([128, QW], F32, tag="rc_b", name="rc_b")
            nc.vector.reciprocal_approx_fast(out=rc_b, in_=b_raw)
            nc.vector.tensor_mul(ot[hp][qh], o_t, rc_b)

        steps = [(qh, hp, j)
                 for qh in range(QH) for hp in range(H // 2)
                 for j in range(nkc)]
        s_pend = {}
        s_pend[0] = emit_S(*steps[0])
        s_pend[1] = emit_S(*steps[1])
        hp_state = {}
        for idx, (qh, hp, j) in enumerate(steps):
            if j == 0:
                o_t = ps.tile([128, QW], F32, tag="o", bufs=2, name="o_t")
                rs_t = rspool.tile([128, 2, 512], BF16, tag="rs",
                                   name="rs_t")
                hp_state[(qh, hp)] = (o_t, rs_t)
            o_t, rs_t = hp_state[(qh, hp)]
            s_t = s_pend.pop(idx)
            pt_t = ppool.tile([128, 2, 512], BF16, tag="pt", name="pt_t")
            nc.scalar.activation(pt_t, s_t, Exp,
                                 bias=kb_t[:, j:j + 1], scale=SCALE)
            if idx + 2 < len(steps):
                s_pend[idx + 2] = emit_S(*steps[idx + 2])
            if j == 0:
                nc.vector.tensor_copy(rs_t, pt_t)
            else:
                nc.vector.tensor_add(rs_t, rs_t, pt_t)
            for a in range(2):
                h = 2 * hp + a
                nc.tensor.matmul(
                    o_t[a * 64:(a + 1) * 64, :],
                    v_nat[j][:, h * D:(h + 1) * D],
                    pt_t[:, a, :],
                    start=(j == 0), stop=(j == nkc - 1),
                    tile_position=(0, a * 64))
            if j == nkc - 1:
                epilogue(qh, hp, rs_t, o_t)
                del hp_state[(qh, hp)]
                if hp == H // 2 - 1:
                    queue_proj(qh)
            drain(2)
        drain(len(backlog))


_CACHE = {}


def _get_compiled(nkc):
    if nkc in _CACHE:
        return _CACHE[nkc]
    NK = nkc * 128
    nc = bacc.Bacc("TRN2", target_bir_lowering=False, debug=False,
                   num_devices=B)
    x_d = nc.dram_tensor("x", [N, DIM], F32, kind="ExternalInput").ap()
    ki_d = nc.dram_tensor("kidx", [NK], I32, kind="ExternalInput").ap()
    kb_d = nc.dram_tensor("kbias", [NK], F32, kind="ExternalInput").ap()
    wqkv_d = nc.dram_tensor("w_qkv", [DIM, 3 * DIM], F32,
                            kind="ExternalInput").ap()
    wp_d = nc.dram_tensor("w_proj", [DIM, DIM], F32,
                          kind="ExternalInput").ap()
    bp_d = nc.dram_tensor("b_proj", [DIM], F32, kind="ExternalInput").ap()
    o_d = nc.dram_tensor("out", [N, DIM], F32, kind="ExternalOutput").ap()
    with tile.TileContext(nc) as tc:
        _build(nc, tc, (x_d, ki_d, kb_d, wqkv_d, wp_d, bp_d, o_d), nkc)
    nc.compile()
    _CACHE[nkc] = nc
    return nc


def prep_run(x, mask, w_qkv, w_proj, b_proj):
    """Build the compiled program + per-core input maps."""
    x = np.ascontiguousarray(np.asarray(x, dtype=np.float32))
    mask = np.ascontiguousarray(np.asarray(mask, dtype=np.int32))
    w_qkv = np.ascontiguousarray(np.asarray(w_qkv, dtype=np.float32))
    w_proj = np.ascontiguousarray(np.asarray(w_proj, dtype=np.float32))
    b_proj = np.ascontiguousarray(np.asarray(b_proj, dtype=np.float32))

    idxs = [np.flatnonzero(mask[b]).astype(np.int32) for b in range(B)]
    max_valid = max(len(i) for i in idxs)
    nkc = min(NCH, max(1, -(-max_valid // 128)))
    NK = nkc * 128
    kidx = np.zeros((B, NK), dtype=np.int32)
    kbias = np.full((B, NK), -1.0e30, dtype=np.float32)
    for b in range(B):
        n = len(idxs[b])
        kidx[b, :n] = idxs[b]
        kbias[b, :n] = 0.0

    nc = _get_compiled(nkc)
    in_maps = [
        {"x": x[b], "kidx": kidx[b], "kbias": kbias[b], "w_qkv": w_qkv,
         "w_proj": w_proj, "b_proj": b_proj}
        for b in range(B)
    ]
    return nc, in_maps


def kernel(x, mask, w_qkv, w_proj, b_proj):
    nc, in_maps = prep_run(x, mask, w_qkv, w_proj, b_proj)
    last_err = None
    for _ in range(3):
        try:
            res = run_bass_kernel_spmd(nc, in_maps, list(range(B))).results
            return np.stack([res[b]["out"] for b in range(B)], axis=0)
        except Exception as e:  # transient device hiccup: retry
            last_err = e
    raise last_err


# revision 20
# speedup vs baseline: 1.4043x; 1.0036x over previous
"""Trainium2 Bass kernel: masked multi-head self-attention block.

out = softmax_mask((x @ Wq) (x @ Wk)^T / sqrt(d)) (x @ Wv) @ Wp + b

Sharding: data-parallel over batch B=8 across the 8 NeuronCores (one
batch row per core); weights replicated; no collectives.

Key compaction: masked keys contribute exactly zero, so each core
gathers only the valid key rows of x (padded to a 128 multiple; padded
slots get a -1e30 score bias -> exp = 0). K/V and attention run on
NK ~= 1152 keys instead of 2048.

v5: every matmul in the attention stream is the SAME 128x128 PE
tiling mode, because mode switches drain the systolic array (HW
measured: pure-mode streams run ~150 ns/matmul vs ~198 alternating):
  - S matmuls contract over 128 with zero-padded K^T: ktz[hp][a] has
    the head's 64 d-rows in the partitions matching qt and zeros
    elsewhere, so K=128 and the full qt slice streams through.
  - PV matmuls are M=128 with zero-padded V columns; the pair shares
    one zero block (layout [v0 | 0 | v1], slices cols 0:128 / 64:192).
    Only the very first PV of a block uses start=True.
  - Denominator matmuls are M=128 with a ones-column stationary.
Plus the v4 structure: short prelude (transposes, K0, V, Q0), K1-5 /
Q1-5 / proj drained through the attention stream from a backlog into
the spare PSUM slot, flat software-pipelined step stream, reciprocal
after broadcast.
"""
import numpy as np

import concourse.bass as bass
import concourse.tile as tile
from concourse import bacc, mybir
from concourse.bass_utils import run_bass_kernel_spmd
from concourse.masks import make_identity

F32 = mybir.dt.float32
BF16 = mybir.dt.bfloat16
I32 = mybir.dt.int32

B, N, DIM = 8, 2048, 768
H, D = 12, 64
SCALE = D ** -0.5
NCH = N // 128        # 16 token chunks
KCH = DIM // 128      # 6 feature chunks
QH = 4                # query blocks
QW = N // QH          # 512 queries per block
Exp = mybir.ActivationFunctionType.Exp


def _nslices(w):
    out = [512] * (w // 512)
    if w % 512:
        out.append(w % 512)
    return out


def _build(nc, tc, aps, nkc):
    x_d, ki_d, kb_d, wqkv_d, wp_d, bp_d, o_d = aps
    NK = nkc * 128

    cpool = tc.alloc_tile_pool(name="const", bufs=1)
    ident = cpool.tile([128, 128], BF16)
    make_identity(nc, ident)
    ones_c = cpool.tile([128, 128], BF16)
    nc.vector.memset(ones_c, 0.0)
    nc.vector.memset(ones_c[:, 0:1], 1.0)
    kb_t = cpool.tile([128, nkc], F32)
    nc.sync.dma_start(out=kb_t, in_=kb_d.rearrange("(j p) -> p j", p=128))
    ki_t = cpool.tile([128, nkc], I32)
    nc.sync.dma_start(out=ki_t, in_=ki_d.rearrange("(j p) -> p j", p=128))
    bp_bc = cpool.tile([128, DIM], F32)
    bp_ap = bass.AP(tensor=bp_d.tensor, offset=bp_d.offset,
                    ap=[[0, 128], list(bp_d.ap[0])])
    nc.sync.dma_start(out=bp_bc, in_=bp_ap)

    # persistent tiles (live through attention; released at the end)
    qkvpool = tc.alloc_tile_pool(name="qkv_sb", bufs=1)
    qt = [qkvpool.tile([128, N], BF16, tag=f"qt{m}", name=f"qt{m}")
          for m in range(KCH)]
    # ktz[hp][a]: zero-padded K^T; head rows live in partitions
    # a*64..a*64+63, other 64 partitions are zero
    ktz = [[qkvpool.tile([128, NK], BF16, tag=f"ktz{m}_{a}",
                         name=f"ktz{m}_{a}") for a in range(2)]
           for m in range(KCH)]
    # vz[t]: per head pair [v_even | zeros | v_odd] (192 cols per pair)
    vz = [qkvpool.tile([128, KCH, 192], BF16, tag=f"vz{t}", name=f"vz{t}")
          for t in range(nkc)]
    wp_sb = [qkvpool.tile([128, DIM], BF16, tag=f"wp{c}", name=f"wp{c}")
             for c in range(KCH)]
    # ot ring over 2 query blocks (proj of qh drains during qh+1)
    ot = [[qkvpool.tile([128, QW], BF16, tag=f"ot{c}_{q}", name=f"ot{c}_{q}")
           for q in range(2)] for c in range(KCH)]
    xt = [qkvpool.tile([128, N], BF16, tag=f"xt{c}", name=f"xt{c}")
          for c in range(KCH)]
    xct = [qkvpool.tile([128, NK], BF16, tag=f"xct{c}", name=f"xct{c}")
           for c in range(KCH)]
    wq_sb = [qkvpool.tile([128, 3 * DIM], BF16, tag=f"wq{c}", name=f"wq{c}")
             for c in range(KCH)]

    # zero padding (written once)
    for m in range(KCH):
        nc.gpsimd.memset(ktz[m][0][64:128, :], 0.0)
        nc.gpsimd.memset(ktz[m][1][0:64, :], 0.0)
    for t in range(nkc):
        nc.vector.memset(vz[t][:, :, 64:128], 0.0)

    with tc.tile_pool(name="stage_sb", bufs=3) as spool:
        # gathers alone on gpsimd queue; x loads on sync; weights on
        # scalar
        gath = []
        for t in range(nkc):
            xg = spool.tile([128, DIM], F32, tag="xg", name=f"xg{t}",
                            bufs=3)
            nc.gpsimd.indirect_dma_start(
                out=xg, out_offset=None, in_=x_d,
                in_offset=bass.IndirectOffsetOnAxis(
                    ap=ki_t[:, t:t + 1], axis=0))
            gath.append(xg)
        wstage = []
        for c in range(KCH):
            ws = spool.tile([128, 3 * DIM], F32, tag="wstage",
                            name=f"ws{c}", bufs=2)
            nc.scalar.dma_start(out=ws, in_=wqkv_d[c * 128:(c + 1) * 128, :])
            wstage.append(ws)
        loads = []
        for ti in range(NCH):
            xs = spool.tile([128, DIM], F32, tag="xs", name=f"xs{ti}",
                            bufs=3)
            nc.sync.dma_start(out=xs, in_=x_d[ti * 128:(ti + 1) * 128, :])
            loads.append(xs)

        with tc.tile_pool(name="ps_a", bufs=4, space="PSUM") as ps_a:

            def transpose_chunk(src, dst, col, i):
                xb = spool.tile([128, DIM], BF16, tag="xb", name=f"xb{i}",
                                bufs=3)
                nc.scalar.copy(xb, src)
                for c in range(KCH):
                    tp = ps_a.tile([128, 128], BF16, tag="tp", name="tp")
                    nc.tensor.transpose(
                        tp, xb[:, c * 128:(c + 1) * 128], ident)
                    nc.vector.tensor_copy(
                        dst[c][:, col * 128:(col + 1) * 128], tp)

            # gathered chunks -> xct; weight casts interleaved on DVE
            for t in range(nkc):
                transpose_chunk(gath[t], xct, t, t)
                if 2 <= t < 2 + KCH:
                    nc.vector.tensor_copy(wq_sb[t - 2], wstage[t - 2])
            for c in range(max(0, nkc - 2), KCH):
                nc.vector.tensor_copy(wq_sb[c], wstage[c])

            # K^T chunk 0 (needed by the first attention steps)
            with tc.tile_pool(name="ps_k", bufs=1, space="PSUM") as ps_k:
                mm_ps = ps_k.tile([128, NK], F32, tag="k_ps", name="k_ps")
                for c in range(KCH):
                    off = 0
                    for w in _nslices(NK):
                        nc.tensor.matmul(
                            mm_ps[:, off:off + w],
                            wq_sb[c][:, DIM:DIM + 128],
                            xct[c][:, off:off + w],
                            start=(c == 0), stop=(c == KCH - 1))
                        off += w
                nc.scalar.copy(ktz[0][0][0:64, :], mm_ps[0:64, :])
                nc.scalar.copy(ktz[0][1][64:128, :], mm_ps[64:128, :])
            # V into the zero-padded pair layout
            with tc.tile_pool(name="ps_v", bufs=2, space="PSUM") as ps_v:
                for t in range(nkc):
                    v_ps = ps_v.tile([128, 2, 512], F32, tag="v_ps",
                                     name="v_ps")
                    for c in range(KCH):
                        nc.tensor.matmul(
                            v_ps[:, 0, :],
                            xct[c][:, t * 128:(t + 1) * 128],
                            wq_sb[c][:, 2 * DIM:2 * DIM + 512],
                            start=(c == 0), stop=(c == KCH - 1))
                        nc.tensor.matmul(
                            v_ps[:, 1, 0:256],
                            xct[c][:, t * 128:(t + 1) * 128],
                            wq_sb[c][:, 2 * DIM + 512:3 * DIM],
                            start=(c == 0), stop=(c == KCH - 1))
                    # head 2hp+a of v_ps -> vz[t][:, hp, a*128 : a*128+64]
                    v0 = v_ps[:, 0, :].rearrange("p (h a d) -> p a h d",
                                                 a=2, d=64)
                    v1 = v_ps[:, 1, 0:256].rearrange(
                        "p (h a d) -> p a h d", a=2, d=64)
                    nc.vector.tensor_copy(vz[t][:, 0:4, 0:64], v0[:, 0])
                    nc.vector.tensor_copy(vz[t][:, 0:4, 128:192], v0[:, 1])
                    nc.vector.tensor_copy(vz[t][:, 4:6, 0:64], v1[:, 0])
                    nc.vector.tensor_copy(vz[t][:, 4:6, 128:192], v1[:, 1])
            # full x chunks -> xt; wp loads+casts interleaved
            for c in range(KCH):
                ws = spool.tile([128, DIM], F32, tag="wpstage",
                                name=f"wps{c}", bufs=2)
                nc.scalar.dma_start(out=ws,
                                    in_=wp_d[c * 128:(c + 1) * 128, :])
                nc.vector.tensor_copy(wp_sb[c], ws)
            for ti in range(NCH):
                transpose_chunk(loads[ti], xt, ti, nkc + ti)
            # Q^T chunk 0
            with tc.tile_pool(name="ps_q0", bufs=1, space="PSUM") as ps_q0:
                mm_ps = ps_q0.tile([128, N], F32, tag="q_ps", name="q_ps")
                for c in range(KCH):
                    for g in range(N // 512):
                        nc.tensor.matmul(
                            mm_ps[:, g * 512:(g + 1) * 512],
                            wq_sb[c][:, 0:128],
                            xt[c][:, g * 512:(g + 1) * 512],
                            start=(c == 0), stop=(c == KCH - 1))
                nc.scalar.copy(qt[0], mm_ps)

    # deferred K^T/Q^T chunk closures (run inside attention stream,
    # borrowing the proj PSUM slot)
    def qk_backlog(ps):
        items = []
        for m in range(1, KCH):
            for lo, hi in [(0, 512), (512, NK)]:
                st = {}

                def start_k(st=st, w=hi - lo):
                    st["ps"] = ps.tile([128, w], F32, tag="pr", bufs=1,
                                       name="kq_ps")

                def kstep(cs, st=st, m=m, lo=lo, hi=hi):
                    for c in cs:
                        off = 0
                        for w in _nslices(hi - lo):
                            nc.tensor.matmul(
                                st["ps"][:, off:off + w],
                                wq_sb[c][:, DIM + m * 128:
                                         DIM + (m + 1) * 128],
                                xct[c][:, lo + off:lo + off + w],
                                start=(c == 0), stop=(c == KCH - 1))
                            off += w

                def fin_k(st=st, m=m, lo=lo, hi=hi):
                    nc.vector.tensor_copy(ktz[m][0][0:64, lo:hi],
                                          st["ps"][0:64, :])
                    nc.vector.tensor_copy(ktz[m][1][64:128, lo:hi],
                                          st["ps"][64:128, :])

                items.append(start_k)
                items.append(lambda f=kstep: f((0, 1, 2)))
                items.append(lambda f=kstep: f((3, 4, 5)))
                items.append(fin_k)
            # Q^T[m] halves
            for half in range(2):
                st = {}
                lo = half * 1024

                def start_q(st=st):
                    st["ps"] = ps.tile([128, 1024], F32, tag="pr",
                                       bufs=1, name="kq_ps")

                def qstep(cs, st=st, m=m, lo=lo):
                    for c in cs:
                        for g in range(2):
                            nc.tensor.matmul(
                                st["ps"][:, g * 512:(g + 1) * 512],
                                wq_sb[c][:, m * 128:(m + 1) * 128],
                                xt[c][:, lo + g * 512:
                                      lo + (g + 1) * 512],
                                start=(c == 0), stop=(c == KCH - 1))

                def fin_q(st=st, m=m, lo=lo):
                    nc.vector.tensor_copy(qt[m][:, lo:lo + 1024],
                                          st["ps"])

                items.append(start_q)
                items.append(lambda f=qstep: f((0, 1)))
                items.append(lambda f=qstep: f((2, 3)))
                items.append(lambda f=qstep: f((4, 5)))
                items.append(fin_q)
        return items

    # ---------------- attention + proj --------------------------
    _attention(nc, tc, qt, ktz, vz, kb_t, ones_c, ot, nkc, wp_sb,
               bp_bc, o_d, qk_backlog)
    qkvpool.release()
    cpool.release()


def _attention(nc, tc, qt, ktz, vz, kb_t, ones_c, ot, nkc,
               wp, bp_bc, o_d, qk_backlog):
    with tc.tile_pool(name="p_sb", bufs=3) as ppool, \
         tc.tile_pool(name="rs_sb", bufs=2) as rspool, \
         tc.tile_pool(name="ep_sb", bufs=2) as eppool, \
         tc.tile_pool(name="out_sb", bufs=3) as outpool, \
         tc.tile_pool(name="dr_sb", bufs=3, space="DRAM") as drpool, \
         tc.tile_pool(name="ps_c", bufs=1, space="PSUM") as ps:

        backlog = qk_backlog(ps)

        def drain(k):
            for _ in range(min(k, len(backlog))):
                backlog.pop(0)()

        def emit_S(qh, hp, j):
            q0 = qh * QW
            s_t = ps.tile([128, 2, 512], F32, tag="s", bufs=2, name="s_t")
            for a in range(2):
                nc.tensor.matmul(
                    s_t[:, a, :],
                    ktz[hp][a][:, j * 128:(j + 1) * 128],
                    qt[hp][:, q0:q0 + QW],
                    start=True, stop=True)
            return s_t

        def queue_proj(qh):
            tags = ("pr",) if qh < QH - 1 else ("s", "pr")
            for ti in range(4):
                t_i = qh * 4 + ti
                st = {}
                tag = tags[ti % len(tags)]

                def start_chunk(st=st, tag=tag):
                    if tag == "s":
                        st["pr"] = ps.tile([128, 2, 512], F32, tag="s",
                                           bufs=2, name="pr")
                    else:
                        st["pr"] = ps.tile([128, 2, 512], F32, tag="pr",
                                           bufs=1, name="pr")

                def cstep(c, t_i=t_i, st=st):
                    tl = (t_i % 4) * 128
                    pr = st["pr"]
                    nc.tensor.matmul(
                        pr[:, 0, :], ot[c][(t_i // 4) % 2][:, tl:tl + 128],
                        wp[c][:, 0:512],
                        start=(c == 0), stop=(c == KCH - 1))
                    nc.tensor.matmul(
                        pr[:, 1, 0:256], ot[c][(t_i // 4) % 2][:, tl:tl + 128],
                        wp[c][:, 512:DIM],
                        start=(c == 0), stop=(c == KCH - 1))

                def finish(t_i=t_i, st=st):
                    pr = st["pr"]
                    out_t = outpool.tile([128, DIM], F32, tag="out_t",
                                         name="out_t")
                    nc.vector.tensor_add(out_t[:, 0:512], pr[:, 0, :],
                                         bp_bc[:, 0:512])
                    nc.vector.tensor_add(out_t[:, 512:DIM], pr[:, 1, 0:256],
                                         bp_bc[:, 512:DIM])
                    nc.sync.dma_start(
                        out=o_d[t_i * 128:(t_i + 1) * 128, :], in_=out_t)

                backlog.append(start_chunk)
                for c in range(KCH):
                    backlog.append(lambda c=c, f=cstep: f(c))
                backlog.append(finish)

        def epilogue(qh, hp, rs_t, o_t):
            # dn row 0 = column sums of rs (ones-column stationary)
            dn_t = ps.tile([128, 2, 512], F32, tag="s", bufs=2, name="dn_t")
            for a in range(2):
                nc.tensor.matmul(dn_t[:, a, :], ones_c, rs_t[:, a, :],
                                 start=True, stop=True)
            dn_sb = eppool.tile([1, 2, 512], F32, tag="dn_sb", name="dn_sb")
            nc.vector.tensor_copy(dn_sb, dn_t[0:1, :, :])
            rc_dram = drpool.tile([1024], F32, tag="rc_dram", name="rc_dram")
            nc.sync.dma_start(out=rc_dram, in_=dn_sb)
            b_raw = eppool.tile([128, QW], F32, tag="b_raw", name="b_raw",
                                bufs=3)
            for a in range(2):
                bc_ap = bass.AP(
                    tensor=rc_dram.tensor,
                    offset=rc_dram.offset + a * 512,
                    ap=[[0, 64], [1, 512]])
                nc.sync.dma_start(out=b_raw[a * 64:(a + 1) * 64, :],
                                  in_=bc_ap)
            rc_b = eppool.tile([128, QW], F32, tag="rc_b", name="rc_b",
                               bufs=3)
            nc.vector.reciprocal_approx_fast(out=rc_b, in_=b_raw)
            nc.vector.tensor_mul(ot[hp][qh % 2], o_t, rc_b)

        steps = [(qh, hp, j)
                 for qh in range(QH) for hp in range(H // 2)
                 for j in range(nkc)]
        s_pend = {}
        s_pend[0] = emit_S(*steps[0])
        s_pend[1] = emit_S(*steps[1])
        hp_state = {}
        for idx, (qh, hp, j) in enumerate(steps):
            if j == 0:
                o_t = ps.tile([128, QW], F32, tag="o", bufs=2, name="o_t")
                rs_t = rspool.tile([128, 2, 512], BF16, tag="rs",
                                   name="rs_t")
                hp_state[(qh, hp)] = (o_t, rs_t)
            o_t, rs_t = hp_state[(qh, hp)]
            s_t = s_pend.pop(idx)
            pt_t = ppool.tile([128, 2, 512], BF16, tag="pt", name="pt_t")
            nc.scalar.activation(pt_t, s_t, Exp,
                                 bias=kb_t[:, j:j + 1], scale=SCALE)
            if idx + 2 < len(steps):
                s_pend[idx + 2] = emit_S(*steps[idx + 2])
            if j == 0:
                nc.vector.tensor_copy(rs_t, pt_t)
            else:
                nc.vector.tensor_add(rs_t, rs_t, pt_t)
            for a in range(2):
                nc.tensor.matmul(
                    o_t[:, :],
                    vz[j][:, hp, a * 64:a * 64 + 128],
                    pt_t[:, a, :],
                    start=(j == 0 and a == 0), stop=(j == nkc - 1 and a == 1))
            if j == nkc - 1:
                epilogue(qh, hp, rs_t, o_t)
                del hp_state[(qh, hp)]
                if hp == H // 2 - 1:
                    queue_proj(qh)
            drain(3 if idx < 36 else 2)
        drain(len(backlog))


_CACHE = {}


def _get_compiled(nkc):
    if nkc in _CACHE:
        return _CACHE[nkc]
    NK = nkc * 128
    nc = bacc.Bacc("TRN2", target_bir_lowering=False, debug=False,
                   num_devices=B)
    x_d = nc.dram_tensor("x", [N, DIM], F32, kind="ExternalInput").ap()
    ki_d = nc.dram_tensor("kidx", [NK], I32, kind="ExternalInput").ap()
    kb_d = nc.dram_tensor("kbias", [NK], F32, kind="ExternalInput").ap()
    wqkv_d = nc.dram_tensor("w_qkv", [DIM, 3 * DIM], F32,
                            kind="ExternalInput").ap()
    wp_d = nc.dram_tensor("w_proj", [DIM, DIM], F32,
                          kind="ExternalInput").ap()
    bp_d = nc.dram_tensor("b_proj", [DIM], F32, kind="ExternalInput").ap()
    o_d = nc.dram_tensor("out", [N, DIM], F32, kind="ExternalOutput").ap()
    with tile.TileContext(nc) as tc:
        _build(nc, tc, (x_d, ki_d, kb_d, wqkv_d, wp_d, bp_d, o_d), nkc)
    nc.compile()
    _CACHE[nkc] = nc
    return nc


def prep_run(x, mask, w_qkv, w_proj, b_proj):
    """Build the compiled program + per-core input maps."""
    x = np.ascontiguousarray(np.asarray(x, dtype=np.float32))
    mask = np.ascontiguousarray(np.asarray(mask, dtype=np.int32))
    w_qkv = np.ascontiguousarray(np.asarray(w_qkv, dtype=np.float32))
    w_proj = np.ascontiguousarray(np.asarray(w_proj, dtype=np.float32))
    b_proj = np.ascontiguousarray(np.asarray(b_proj, dtype=np.float32))

    idxs = [np.flatnonzero(mask[b]).astype(np.int32) for b in range(B)]
    max_valid = max(len(i) for i in idxs)
    nkc = min(NCH, max(1, -(-max_valid // 128)))
    NK = nkc * 128
    kidx = np.zeros((B, NK), dtype=np.int32)
    kbias = np.full((B, NK), -1.0e30, dtype=np.float32)
    for b in range(B):
        n = len(idxs[b])
        kidx[b, :n] = idxs[b]
        kbias[b, :n] = 0.0

    nc = _get_compiled(nkc)
    in_maps = [
        {"x": x[b], "kidx": kidx[b], "kbias": kbias[b], "w_qkv": w_qkv,
         "w_proj": w_proj, "b_proj": b_proj}
        for b in range(B)
    ]
    return nc, in_maps


def kernel(x, mask, w_qkv, w_proj, b_proj):
    nc, in_maps = prep_run(x, mask, w_qkv, w_proj, b_proj)
    last_err = None
    for _ in range(3):
        try:
            res = run_bass_kernel_spmd(nc, in_maps, list(range(B))).results
            return np.stack([res[b]["out"] for b in range(B)], axis=0)
        except Exception as e:  # transient device hiccup: retry
            last_err = e
    raise last_err
